# revision 32
# baseline (speedup 1.0000x reference)
"""Trainium2 Bass kernel for nn_BinaryTemporalBlock (Conv-TasNet-style binary
temporal block): 1x1 binarized conv (128->512) -> gLN -> PReLU -> dilated
depthwise binarized conv (K=3, dil=4) -> gLN -> PReLU -> two 1x1 binarized
convs (512->128 residual-out and 512->128 skip).

Sharding: data-parallel over batch. B=8 samples on 8 NeuronCores, one sample
per core; gLN is per-sample so no collectives are needed.

Per-core strategy (sample = [C=128, T=4000]):
  - Host binarizes weights: sign matrices (exact +-1 in bf16) go through the
    PE array; per-output-channel alpha scales stay fp32 and ride the
    PSUM->SBUF drains (free affine in ScalarE activation / DVE tensor_scalar).
  - Matmuls fill 4-bank PSUM groups ([128,4,512]); one drain per group with
    accum_out collecting per-channel sums for gLN. Sum-of-squares by a second
    pass (ScalarE Square+accum or DVE scalar_tensor_tensor+accum), engine
    split tuned via CFG.
  - Partition reduce+broadcast of stats via two tiny PE matmuls with ones.
  - norm+PReLU: ScalarE Prelu (scale/bias/alpha) on half the tiles, DVE
    tensor_scalar + max(z, p*z) on the rest (valid for p <= 1).
  - depthwise dilated conv: 3 diagonal-sign matmuls per tile accumulating in
    PSUM (taps at t-4, t, t+4 via shifted APs on a halo-padded tile).
"""

import os
import sys

sys.path.insert(0, "/opt/trn_rl_repo")

import numpy as np
import ml_dtypes

import concourse.bass as bass
import concourse.tile as tile
from concourse import bacc
from concourse import mybir
from concourse.bass_utils import run_bass_kernel_spmd

F32 = mybir.dt.float32
BF16 = mybir.dt.bfloat16
NPBF16 = ml_dtypes.bfloat16
ALU = mybir.AluOpType
AFT = mybir.ActivationFunctionType
AX = mybir.AxisListType

B, C, H, SC, T = 8, 128, 512, 128, 4000
HR = H // 128          # 4 h-rows of 128 partitions
CW = 500               # matmul chunk width (<=512 fp32 PSUM bank)
GB = 4                 # PSUM banks per drain group
GW = GB * CW           # drain-group width
NG = T // GW           # groups per row
DIL = 4
EPS = 1e-8
NTOT = float(H * T)

CFG = {
    "np1_act_rows": (0, 1, 2),      # h1n rows normalized on ScalarE Prelu
    "np2_act_rows": (0, 1, 2, 3),      # h2n rows on ScalarE Prelu
    "sumsq_act_rows": (0, 2),    # rows whose sum-of-squares runs on ScalarE
    "skip_act_pairs": (),    # skip-drain pairs on ScalarE
}

last_run_info = {}


def _binarize(w):
    alpha = np.mean(np.abs(w), axis=tuple(range(1, w.ndim)))
    return alpha.astype(np.float32), np.sign(w).astype(np.float32)


def _cols(v):
    """[512] channel vector -> [128, HR] column-per-h-row layout."""
    return np.ascontiguousarray(v.reshape(HR, 128).T.astype(np.float32))


def _prep(inputs):
    x = np.asarray(inputs["x"], np.float32)
    p1 = float(np.asarray(inputs["p1"]))
    p2 = float(np.asarray(inputs["p2"]))
    b1 = np.asarray(inputs["b1"], np.float32).reshape(-1)
    g1 = np.asarray(inputs["g1"], np.float32).reshape(-1)
    be1 = np.asarray(inputs["be1"], np.float32).reshape(-1)
    g2 = np.asarray(inputs["g2"], np.float32).reshape(-1)
    be2 = np.asarray(inputs["be2"], np.float32).reshape(-1)
    b2 = np.asarray(inputs["b2"], np.float32).reshape(-1)
    bsk = np.asarray(inputs["b_skip"], np.float32).reshape(-1)

    a1, s1 = _binarize(np.asarray(inputs["w1"], np.float32))
    adw, sdw = _binarize(np.asarray(inputs["w_dw"], np.float32))
    a2, s2 = _binarize(np.asarray(inputs["w2"], np.float32))
    ask, ssk = _binarize(np.asarray(inputs["w_skip"], np.float32))
    s1 = s1[:, :, 0]      # [512,128]
    sdw = sdw[:, 0, :]    # [512,3]
    s2 = s2[:, :, 0]      # [128,512]
    ssk = ssk[:, :, 0]

    # one packed bf16 weight tensor: lhsT1 | dwdiag | lhsT2 | lhsTsk
    wcat = np.zeros((128, 24, 128), NPBF16)
    wcat[:, 0:4, :] = s1.T.reshape(128, 4, 128)
    for r in range(HR):
        for k in range(3):
            np.fill_diagonal(wcat[:, 4 + r * 3 + k, :],
                             sdw[r * 128:(r + 1) * 128, k])
    for k in range(HR):
        wcat[:, 16 + k, :] = s2[:, k * 128:(k + 1) * 128].T
        wcat[:, 20 + k, :] = ssk[:, k * 128:(k + 1) * 128].T

    # one packed fp32 param tensor: 7 h-row columns [128,7,4] + 4 C columns
    fpar = np.zeros((128, 32), np.float32)
    hcols = np.stack([_cols(a1), _cols(b1), _cols(g1), _cols(be1),
                      _cols(adw), _cols(g2), _cols(be2)], axis=1)  # [128,7,4]
    fpar[:, 0:28] = hcols.reshape(128, 28)
    fpar[:, 28:32] = np.stack([a2, b2, ask, bsk], axis=1)

    common = {
        "wcat": np.ascontiguousarray(wcat.reshape(128, 24 * 128)),
        "fpar": np.ascontiguousarray(fpar),
    }
    return x, p1, p2, common


def _r3(ap, b=CW):
    """[128, k*b] contiguous slice -> [128, k, b] view."""
    return ap.rearrange("p (a b) -> p a b", b=b)


def _build(p1, p2):
    nc = bacc.Bacc("TRN2", target_bir_lowering=False, debug=False, num_devices=8)
    x_in = nc.declare_dram_parameter("x_in", [C, T], F32, False)
    wcat_in = nc.declare_dram_parameter("wcat", [128, 24 * 128], BF16, False)
    fpar_in = nc.declare_dram_parameter("fpar", [128, 32], F32, False)
    out_r = nc.declare_dram_parameter("out_r", [C, T], F32, True)
    skip_r = nc.declare_dram_parameter("skip_r", [SC, T], F32, True)

    np1_act = CFG["np1_act_rows"]
    np2_act = CFG["np2_act_rows"]
    sq_act = CFG["sumsq_act_rows"]
    skip_act = CFG["skip_act_pairs"]

    with tile.TileContext(nc) as tc:
        with (
            tc.tile_pool(name="persist", bufs=1) as pp,
            tc.tile_pool(name="outp", bufs=3) as outp,
            tc.tile_pool(name="small", bufs=1) as small,
            tc.tile_pool(name="mm", bufs=2, space="PSUM") as mmp,
        ):
            # ---- x first (4 chunks), cast to bf16 on DVE
            x_t = pp.tile([128, T], F32, tag="x")
            xb = pp.tile([128, T], BF16, tag="xb")
            for q in range(4):
                sl = slice(q * 1000, (q + 1) * 1000)
                nc.sync.dma_start(out=x_t[:, sl], in_=x_in[:, sl])
                nc.vector.tensor_copy(out=xb[:, sl], in_=x_t[:, sl])

            # ---- packed weights / params (one DMA each)
            wcat = pp.tile([128, 24, 128], BF16, tag="wcat")
            nc.sync.dma_start(out=wcat[:], in_=_r3(wcat_in[:], 128))
            fpar = pp.tile([128, 32], F32, tag="fpar")
            nc.sync.dma_start(out=fpar[:], in_=fpar_in[:])
            hp = _r3(fpar[:, 0:28], HR)                   # [128,7,4]
            a1c, b1c, g1c, be1c, adwc, g2c, be2c = (hp[:, i, :] for i in range(7))
            a2c, b2c, askc, bskc = (fpar[:, 28 + i:29 + i] for i in range(4))
            w1s = wcat[:, 0:4, :]
            dws = wcat[:, 4:16, :]
            w2s = wcat[:, 16:20, :]
            wss = wcat[:, 20:24, :]
            eps_t = small.tile([128, 1], F32, tag="eps")
            nc.vector.memset(eps_t[:], EPS)
            ones_c = small.tile([128, 1], F32, tag="ones_c")
            nc.vector.memset(ones_c[:], 1.0)
            ones_r = small.tile([1, 128], F32, tag="ones_r")
            nc.vector.memset(ones_r[:], 1.0)

            # residual base x2 = x + b2 on DVE (also absorbs the fpar DMA
            # tick on DVE before the drains need it)
            x2_t = pp.tile([128, T], F32, tag="x2")
            for q in range(4):
                sl = slice(q * 1000, (q + 1) * 1000)
                nc.vector.tensor_scalar(out=x2_t[:, sl], in0=x_t[:, sl],
                                        scalar1=b2c, scalar2=None, op0=ALU.add)

            scr_d = pp.tile([128, T], BF16, tag="scr_d")    # DVE-only scratch
            scr_a = pp.tile([128, T], BF16, tag="scr_a")    # ACT-only scratch
            h1 = [pp.tile([128, T], BF16, tag=f"hbig_{r}", name=f"h1_{r}")
                  for r in range(HR)]
            h1n = [pp.tile([128, T + 2 * DIL], BF16, tag=f"h1n_{r}", name=f"h1n_{r}")
                   for r in range(HR)]
            h2 = [pp.tile([128, T], BF16, tag=f"h2_{r}", name=f"h2_{r}")
                  for r in range(HR)]
            h2n = [pp.tile([128, T], BF16, tag=f"hbig_{r}", name=f"h2n_{r}")
                   for r in range(HR)]

            st1 = small.tile([128, 2 * HR * NG + HR], F32, tag="st1")
            st2 = small.tile([128, 2 * HR * NG + HR], F32, tag="st2")
            scb1 = small.tile([128, 2 * HR], F32, tag="scb1")
            scb2 = small.tile([128, 2 * HR], F32, tag="scb2")
            ws1 = small.tile([128, 8], F32, tag="ws1")
            ws2 = small.tile([128, 8], F32, tag="ws2")

            def drain(gidx, ps4, dst2000, scale_col, bias_col, st):
                """Empty a 4-bank PSUM group with per-channel affine and
                accumulate per-channel sums. Engine by gidx parity (aligned
                with the 2-slot PSUM cycle so slot WAW stays same-engine)."""
                pview = ps4[:, 0:GB, 0:CW]
                oview = _r3(dst2000)
                if gidx % 2 == 0:
                    nc.scalar.activation(oview, pview, AFT.Identity,
                                         bias=bias_col, scale=scale_col,
                                         accum_out=st[:, gidx:gidx + 1])
                else:
                    nc.vector.tensor_scalar(out=oview, in0=pview,
                                            scalar1=scale_col, scalar2=bias_col,
                                            op0=ALU.mult, op1=ALU.add,
                                            accum_out=st[:, gidx:gidx + 1])

            def sumsq(r, src, st):
                if r in sq_act:
                    nc.scalar.activation(scr_a[:, 0:T], src, AFT.Square,
                                         accum_out=st[:, 2 * HR * NG + r:2 * HR * NG + r + 1])
                else:
                    nc.vector.scalar_tensor_tensor(
                        out=scr_d[:, 0:T], in0=src, scalar=0.0, in1=src,
                        op0=ALU.bypass, op1=ALU.mult,
                        accum_out=st[:, 2 * HR * NG + r:2 * HR * NG + r + 1])

            # ---- block 1: conv1 (sign matmuls) + drains
            for r in range(HR):
                for g in range(NG):
                    ps4 = mmp.tile([128, GB, 512], F32, tag="mm4",
                                   name=f"c1ps_{r}_{g}")
                    for c4 in range(GB):
                        cc = g * GB + c4
                        nc.tensor.matmul(ps4[:, c4, 0:CW], w1s[:, r, :],
                                         xb[:, cc * CW:(cc + 1) * CW],
                                         start=True, stop=True)
                    drain(r * NG + g, ps4, h1[r][:, g * GW:(g + 1) * GW],
                          a1c[:, r:r + 1], b1c[:, r:r + 1], st1)
            for r in range(HR):
                sumsq(r, h1[r][:], st1)

            def stats_join(st, scb, gcol, becol, ws, tagsfx):
                nsum = 2 * HR * NG
                nc.vector.reduce_sum(out=ws[:, 0:1], in_=st[:, 0:nsum], axis=AX.X)
                nc.vector.reduce_sum(out=ws[:, 1:2], in_=st[:, nsum:nsum + HR],
                                     axis=AX.X)
                ps_t = mmp.tile([1, 2], F32, tag="mm4", name=f"pst_{tagsfx}")
                nc.tensor.matmul(ps_t[:], ones_c[:], ws[:, 0:2],
                                 start=True, stop=True)
                st_s = small.tile([1, 2], F32, tag=f"sts_{tagsfx}")
                nc.vector.tensor_copy(out=st_s[:], in_=ps_t[:])
                ps_b = mmp.tile([128, 2], F32, tag="mm4", name=f"psb_{tagsfx}")
                nc.tensor.matmul(ps_b[:], ones_r[:], st_s[:],
                                 start=True, stop=True)
                nc.vector.tensor_copy(out=ws[:, 0:2], in_=ps_b[:])
                nc.scalar.activation(ws[:, 2:4], ws[:, 0:2], AFT.Identity,
                                     scale=1.0 / NTOT)
                nc.scalar.activation(ws[:, 4:5], ws[:, 0:1], AFT.Identity,
                                     scale=-1.0 / NTOT)
                nc.vector.tensor_tensor(out=ws[:, 5:6], in0=ws[:, 2:3],
                                        in1=ws[:, 2:3], op=ALU.mult)
                nc.vector.tensor_tensor(out=ws[:, 5:6], in0=ws[:, 3:4],
                                        in1=ws[:, 5:6], op=ALU.subtract)
                nc.scalar.activation(ws[:, 6:7], ws[:, 5:6], AFT.Sqrt,
                                     bias=eps_t[:, 0:1])
                nc.vector.reciprocal(ws[:, 7:8], ws[:, 6:7])
                nc.vector.tensor_scalar_mul(out=scb[:, 0:HR], in0=gcol,
                                            scalar1=ws[:, 7:8])
                nc.vector.scalar_tensor_tensor(out=scb[:, HR:2 * HR],
                                               in0=scb[:, 0:HR],
                                               scalar=ws[:, 4:5], in1=becol,
                                               op0=ALU.mult, op1=ALU.add)

            stats_join(st1, scb1, g1c, be1c, ws1, "1")

            # ---- np: prelu(scale*h+bias); ScalarE Prelu or DVE 3-op form
            def np_unit(on_act, src, dst, sc_col, bi_col, pval):
                if on_act:
                    nc.scalar.activation(dst, src, AFT.Prelu,
                                         bias=bi_col, scale=sc_col, alpha=pval)
                else:
                    w = src.shape[-1]
                    z = scr_d[:, 0:w]
                    pz = scr_d[:, w:2 * w]
                    nc.vector.tensor_scalar(out=z, in0=src, scalar1=sc_col,
                                            scalar2=bi_col,
                                            op0=ALU.mult, op1=ALU.add)
                    nc.vector.tensor_scalar_mul(out=pz, in0=z, scalar1=pval)
                    nc.vector.tensor_tensor(out=dst, in0=z, in1=pz, op=ALU.max)

            for r in range(HR):
                on_act = r in np1_act
                if on_act:
                    nc.scalar.activation(h1n[r][:, 0:DIL], fpar[:, 0:DIL],
                                         AFT.Identity, bias=0.0, scale=0.0)
                    nc.scalar.activation(h1n[r][:, DIL + T:], fpar[:, 0:DIL],
                                         AFT.Identity, bias=0.0, scale=0.0)
                else:
                    nc.vector.memset(h1n[r][:, 0:DIL], 0)
                    nc.vector.memset(h1n[r][:, DIL + T:], 0)
                for u in range(T // 2000):
                    sl = slice(u * 2000, (u + 1) * 2000)
                    np_unit(on_act, h1[r][:, sl],
                            h1n[r][:, DIL + u * 2000:DIL + (u + 1) * 2000],
                            scb1[:, r:r + 1], scb1[:, HR + r:HR + r + 1], p1)

            # ---- block 2: depthwise dilated conv (diag sign matmuls)
            for r in range(HR):
                for g in range(NG):
                    ps4 = mmp.tile([128, GB, 512], F32, tag="mm4",
                                   name=f"dwps_{r}_{g}")
                    for k in range(3):
                        off = (k - 1) * DIL
                        for c4 in range(GB):
                            cc = g * GB + c4
                            st_ = DIL + cc * CW + off
                            nc.tensor.matmul(ps4[:, c4, 0:CW],
                                             dws[:, r * 3 + k, :],
                                             h1n[r][:, st_:st_ + CW],
                                             start=(k == 0), stop=(k == 2))
                    drain(r * NG + g, ps4, h2[r][:, g * GW:(g + 1) * GW],
                          adwc[:, r:r + 1], 0.0, st2)
            for r in range(HR):
                sumsq(r, h2[r][:], st2)

            stats_join(st2, scb2, g2c, be2c, ws2, "2")

            # ---- np2 (pair-major so the final matmuls start early)
            for pr in range(4):
                sl = slice(pr * 1000, (pr + 1) * 1000)
                for r in range(HR):
                    np_unit(r in np2_act, h2[r][:, sl], h2n[r][:, sl],
                            scb2[:, r:r + 1], scb2[:, HR + r:HR + r + 1], p2)

            # ---- finals: out (+residual) and skip per 1000-wide pair
            for pr in range(4):
                ps4 = mmp.tile([128, 4, 512], F32, tag="mm4", name=f"fin_{pr}")
                korder = [3, 0, 1, 2]  # start on a DVE-written h2n row
                for ki, k in enumerate(korder):
                    for j in range(2):
                        cc = pr * 2 + j
                        csl = slice(cc * CW, (cc + 1) * CW)
                        nc.tensor.matmul(ps4[:, j, 0:CW], w2s[:, k, :],
                                         h2n[k][:, csl],
                                         start=(ki == 0), stop=(ki == HR - 1))
                for ki, k in enumerate(korder):
                    for j in range(2):
                        cc = pr * 2 + j
                        csl = slice(cc * CW, (cc + 1) * CW)
                        nc.tensor.matmul(ps4[:, 2 + j, 0:CW], wss[:, k, :],
                                         h2n[k][:, csl],
                                         start=(ki == 0), stop=(ki == HR - 1))
                psl = slice(pr * 1000, (pr + 1) * 1000)
                oc = outp.tile([128, 1000], F32, tag="oc")
                nc.vector.scalar_tensor_tensor(out=_r3(oc[:]),
                                               in0=ps4[:, 0:2, 0:CW],
                                               scalar=a2c,
                                               in1=_r3(x2_t[:, psl]),
                                               op0=ALU.mult, op1=ALU.add)
                nc.sync.dma_start(out=out_r[:, psl], in_=oc[:])
                sc = outp.tile([128, 1000], F32, tag="sc")
                if pr in skip_act:
                    nc.scalar.activation(_r3(sc[:]), ps4[:, 2:4, 0:CW],
                                         AFT.Identity, bias=bskc, scale=askc)
                else:
                    nc.vector.tensor_scalar(out=_r3(sc[:]),
                                            in0=ps4[:, 2:4, 0:CW],
                                            scalar1=askc, scalar2=bskc,
                                            op0=ALU.mult, op1=ALU.add)
                nc.sync.dma_start(out=skip_r[:, psl], in_=sc[:])
    return nc


def _install_ntff_hook():
    """The agent image's antenv lacks axon_hooks; recreate it from the boot
    helper so run_bass_kernel_spmd(trace=True) can capture NTFF profiles."""
    import types
    try:
        from antenv.axon_hooks import get_axon_ntff_profile_hook  # noqa: F401
        return
    except ImportError:
        pass
    try:
        if "/root/.axon_site" not in sys.path:
            sys.path.insert(0, "/root/.axon_site")
        from trn_agent_boot.trn_boot import _ntff_profile_via_ctypes
        hook = _ntff_profile_via_ctypes("/opt/axon/libaxon_pjrt.so")
    except Exception:
        hook = None
    mod = types.ModuleType("antenv.axon_hooks")
    state = {"hook": hook}
    mod.get_axon_ntff_profile_hook = lambda: state["hook"]
    mod.set_axon_ntff_profile_hook = lambda h: state.update(hook=h)
    sys.modules["antenv.axon_hooks"] = mod


def kernel(**inputs):
    x, p1, p2, common = _prep(inputs)
    nc = _build(p1, p2)
    if not nc.is_finalized():
        nc.finalize()
    in_maps = [dict(common, x_in=np.ascontiguousarray(x[b])) for b in range(B)]
    trace = bool(int(os.environ.get("KERNEL_TRACE", "0")))
    if trace:
        _install_ntff_hook()
    res = run_bass_kernel_spmd(nc, in_maps, core_ids=list(range(B)), trace=trace)
    last_run_info.clear()
    last_run_info["exec_time_ns"] = res.exec_time_ns
    last_run_info["results"] = res
    out = np.stack([r["out_r"] for r in res.results]).astype(np.float32)
    skip = np.stack([r["skip_r"] for r in res.results]).astype(np.float32)
    return out, skip
